# revision 25
# baseline (speedup 1.0000x reference)
"""HarmonicMixing Trainium2 kernel.

out[..., k] = x[..., k]
            + sum_s uw_s * x[..., k/s]   for s | k          (up-scatter, s in {2,4,8})
            + sum_s dw_s * P_s[..., k]   for 1 <= k < D/s   (down pooled scatter)
where P_s[k] = sum_{i=k*s}^{(k+1)s-1} x[i] and uw/dw = sigmoid(weights).

Decomposition used on-chip (per 1024-vector, verified vs fp64 ref):
  P2 = x_e + x_o ; P4 = pool(P2) ; P8 = pool(P4)        (adjacent-pair sums)
  T[0::2] = (uw4/uw2)*x[0:256] + x[0:512:2] ; T[0::4] += (uw8/uw2)*x[0:128]
  T[1::2] = x[1:512:2]                                   (T = up-mix / uw2)
  out[512:1024:2] = uw2*T[256:512] + x[512:1024:2] ; out[513:1024:2] = x odds
  out[0]     = (1 + uw2 + uw4 + uw8) * x[0]
  out[1:512] = dw2*P2[1:512] + x[1:512]
  out[1:256] += dw4*P4[1:256] ; out[1:128] += dw8*P8[1:128]
  out[2:512:2] += uw2 * T[1:256]

Engine split: all 2-src ops on DVE (GPSIMD elementwise contends with DVE
for SBUF ports and walrus rejects scalar_tensor_tensor on Pool); 1-src
copies/scales on ScalarE; HWDGE DMAs on nc.sync, hi half stored while the
lo half is still being assembled.

Sharding: pure data-parallel over tokens; batch b -> core b (8 cores x 4096 tokens).
"""

import sys

if "/opt/trn_rl_repo" not in sys.path:
    sys.path.insert(0, "/opt/trn_rl_repo")

import numpy as np

D = 1024
N_CORES = 8
TOK_PER_CORE = 4096
C = 4                      # tokens per partition per iteration
TILE_TOKENS = 128 * C
N_ITERS = TOK_PER_CORE // TILE_TOKENS


def _build(uw, dw):
    import concourse.bacc as bacc
    import concourse.mybir as mybir
    from concourse.tile import TileContext

    f32 = mybir.dt.float32
    MULT = mybir.AluOpType.mult
    ADD = mybir.AluOpType.add

    uw2, uw4, uw8 = [float(v) for v in uw]
    dw2, dw4, dw8 = [float(v) for v in dw]
    r42u = uw4 / uw2
    r82u = uw8 / uw2
    w0 = 1.0 + uw2 + uw4 + uw8

    nc = bacc.Bacc("TRN2", target_bir_lowering=False, debug=False,
                   enable_asserts=False)
    x_d = nc.dram_tensor("x", [TOK_PER_CORE, D], f32, kind="ExternalInput")
    o_d = nc.dram_tensor("o", [TOK_PER_CORE, D], f32, kind="ExternalOutput")

    # partition p of iteration n holds tokens n*512 + p*C + [0, C)
    xv = x_d.ap().rearrange("(n p c) d -> n p c d", p=128, c=C)
    ov = o_d.ap().rearrange("(n p c) d -> n p c d", p=128, c=C)

    H = D // 2  # 512

    with TileContext(nc) as tc:
        with tc.tile_pool(name="xio", bufs=4) as xio, \
             tc.tile_pool(name="oio", bufs=4) as oio, \
             tc.tile_pool(name="wk", bufs=2) as wk, \
             tc.tile_pool(name="tp", bufs=3) as tp:
            for it in range(N_ITERS):
                xt = xio.tile([128, C, D], f32, tag="xt")
                ot = oio.tile([128, C, D], f32, tag="ot")
                p2 = wk.tile([128, C, 512], f32, tag="p2")
                p4 = wk.tile([128, C, 256], f32, tag="p4")
                p8 = wk.tile([128, C, 128], f32, tag="p8")
                tt = tp.tile([128, C, 512], f32, tag="tt")

                nc.sync.dma_start(xt, xv[it])

                # pools on DVE (GPSIMD elementwise contends with DVE for SBUF
                # ports, slowing both); T-odd copy on idle ScalarE
                nc.vector.tensor_add(p2, xt[:, :, 0:D:2], xt[:, :, 1:D:2])
                nc.scalar.copy(tt[:, :, 1:512:2], xt[:, :, 1:H:2])
                nc.vector.tensor_add(p4, p2[:, :, 0:512:2], p2[:, :, 1:512:2])
                nc.vector.tensor_add(p8, p4[:, :, 0:256:2], p4[:, :, 1:256:2])

                # up-mix helper vector T, even part (DVE)
                nc.vector.scalar_tensor_tensor(
                    tt[:, :, 0:512:2], xt[:, :, 0:256], r42u,
                    xt[:, :, 0:H:2], MULT, ADD)
                nc.vector.scalar_tensor_tensor(
                    tt[:, :, 0:512:4], xt[:, :, 0:128], r82u,
                    tt[:, :, 0:512:4], MULT, ADD)

                # hi half of the output first, then store it while the lo
                # half is still being assembled. The tiny out[0] ACT op goes
                # first so IT (on the idle engine) absorbs the ot-slot reuse
                # wait instead of the first DVE op.
                nc.scalar.mul(ot[:, :, 0:1], xt[:, :, 0:1], w0)
                nc.vector.scalar_tensor_tensor(
                    ot[:, :, H:D:2], tt[:, :, 256:512], uw2, xt[:, :, H:D:2],
                    MULT, ADD)
                nc.scalar.copy(ot[:, :, H + 1:D:2], xt[:, :, H + 1:D:2])
                nc.sync.dma_start(ov[it][:, :, H:D], ot[:, :, H:D])

                # lo half
                nc.vector.scalar_tensor_tensor(
                    ot[:, :, 1:H], p2[:, :, 1:512], dw2, xt[:, :, 1:H],
                    MULT, ADD)
                nc.vector.scalar_tensor_tensor(
                    ot[:, :, 1:256], p4[:, :, 1:256], dw4, ot[:, :, 1:256],
                    MULT, ADD)
                nc.vector.scalar_tensor_tensor(
                    ot[:, :, 1:128], p8[:, :, 1:128], dw8, ot[:, :, 1:128],
                    MULT, ADD)
                nc.vector.scalar_tensor_tensor(
                    ot[:, :, 2:H:2], tt[:, :, 1:256], uw2, ot[:, :, 2:H:2],
                    MULT, ADD)
                nc.sync.dma_start(ov[it][:, :, 0:H], ot[:, :, 0:H])

    if not nc.is_finalized():
        nc.finalize()
    return nc


def _run(x, up_weights, down_weights, trace=False):
    from concourse.bass_utils import run_bass_kernel_spmd

    x = np.ascontiguousarray(np.asarray(x, dtype=np.float32))
    uwr = np.asarray(up_weights, dtype=np.float64)
    dwr = np.asarray(down_weights, dtype=np.float64)
    uw = 1.0 / (1.0 + np.exp(-uwr))
    dw = 1.0 / (1.0 + np.exp(-dwr))

    nc = _build(uw, dw)

    orig_shape = x.shape
    xf = x.reshape(N_CORES, TOK_PER_CORE, D)
    in_maps = [{"x": xf[c]} for c in range(N_CORES)]
    res = run_bass_kernel_spmd(nc, in_maps, core_ids=list(range(N_CORES)),
                               trace=trace)
    out = np.stack([res.results[c]["o"] for c in range(N_CORES)], axis=0)
    return out.reshape(orig_shape), res


def kernel(x, up_weights, down_weights):
    out, _ = _run(x, up_weights, down_weights, trace=False)
    return out
